# revision 9
# baseline (speedup 1.0000x reference)
"""Trainium2 Bass kernel for per-8x8-block DCT -> quantize(round) -> IDCT.

Math (per 8x8 spatial block B of x, with orthonormal DCT matrix D):
    X  = D @ B @ D.T          (2D DCT)
    Xq = round(X / q) * q     (quantize / dequantize)
    B' = D.T @ Xq @ D         (2D inverse DCT)

Implementation strategy (per core, data-parallel over the batch axis N):
  - DMA x in as [partition=c(128), free=(8 rows x 112 cols)=896] chunks --
    fully contiguous 3.5KB-per-partition runs, full HBM rate.
  - PE transpose-mode (T1) with a 3D gathered weights-AP moves each pair of
    8x8 blocks into "Kron layout" [partition=(s,j,k) in [0,128), free=c],
    where s indexes the block pair, (j,k) the position inside the block.
  - One 128x128 stationary matmul MM1 = blockdiag(G, G), G = (D kron D),
    computes the full 2D DCT of both blocks in a single pass (fp32).
  - Quantize: round(X/q) == (X*(1/q) + 1.5*2^23) - 1.5*2^23 in fp32 RNE
    arithmetic (exact round-half-even, matching jnp.round).  1/q is a
    per-partition scalar in this layout (partition = in-block position).
    Split across ScalarE (scale+magic-add, PSUM->SBUF) and VectorE
    (subtract magic, cast to fp16 -- the rounded values are small integers,
    exactly representable in fp16).
  - MM2 = blockdiag(G.T diag(q)) in fp16 (1 cycle/row on the PE) computes
    the de-quantized inverse DCT.
  - C2/T2: copy to fp16 SBUF, PE transpose-mode back to [partition=c],
    C3 scatters (j,k) back to the natural (h,w) free-axis layout.
  - DMA out, fully contiguous again.
"""

import os

import numpy as np

N_FULL, C, H, W = 32, 128, 112, 112
N_CORES = 8
N_SHARD = N_FULL // N_CORES  # 4
BLK = 8
HB = H // BLK  # 14
WB = W // BLK  # 14
WBP = WB // 2  # 7 pairs of blocks along W
ROWCHUNK = BLK * W  # 896 floats per (c, hb) chunk
CMAGIC = float(np.float32(1.5 * 2**23))

LAST_EXEC_NS = None
LAST_PROFILE = None


def _dct_mat():
    k = np.arange(BLK, dtype=np.float64)[:, None]
    m = np.arange(BLK, dtype=np.float64)[None, :]
    D = np.cos(np.pi * (2.0 * m + 1.0) * k / (2.0 * BLK))
    D[0, :] *= np.sqrt(1.0 / BLK)
    D[1:, :] *= np.sqrt(2.0 / BLK)
    return D.astype(np.float32)


def _build_consts(q_table: np.ndarray):
    D = _dct_mat()
    # G[(i,l),(j,k)] = D[i,j] * D[l,k]; forward DCT: Xvec = G @ xvec
    G = np.einsum("ij,lk->iljk", D, D).reshape(64, 64).astype(np.float32)
    # MM1 stationary lhsT[(s,jk),(s,il)] = G[(il),(jk)]  (blockdiag over s)
    G1 = np.zeros((128, 128), np.float32)
    G1[:64, :64] = G.T
    G1[64:, 64:] = G.T
    qflat = q_table.reshape(64).astype(np.float32)
    qinv = np.concatenate([1.0 / qflat, 1.0 / qflat]).reshape(128, 1)
    # MM2 stationary lhsT[(s,il),(s,jk)] = G[(il),(jk)] * q[(il)]
    W2s = (G * qflat[:, None]).astype(np.float16)
    W2 = np.zeros((128, 128), np.float16)
    W2[:64, :64] = W2s
    W2[64:, 64:] = W2s
    I32 = np.eye(128, dtype=np.float32)
    I16 = np.eye(128, dtype=np.float16)
    return G1, W2, qinv.astype(np.float32), I32, I16


def _build_program():
    import concourse.mybir as mybir
    from concourse import bacc
    from concourse.tile import TileContext

    fp32 = mybir.dt.float32
    fp16 = mybir.dt.float16

    nc = bacc.Bacc()
    xs = nc.declare_dram_parameter("x", [N_SHARD, C, H, W], fp32, isOutput=False)
    g1 = nc.declare_dram_parameter("g1", [128, 128], fp32, isOutput=False)
    w2 = nc.declare_dram_parameter("w2", [128, 128], fp16, isOutput=False)
    qinv = nc.declare_dram_parameter("qinv", [128, 1], fp32, isOutput=False)
    cmagic = nc.declare_dram_parameter("cmagic", [128, 1], fp32, isOutput=False)
    id32 = nc.declare_dram_parameter("id32", [128, 128], fp32, isOutput=False)
    id16 = nc.declare_dram_parameter("id16", [128, 128], fp16, isOutput=False)
    ys = nc.declare_dram_parameter("y", [N_SHARD, C, H, W], fp32, isOutput=True)

    # wbp groups per hb chunk: [0,4) -> 512 cols, [4,7) -> 384 cols
    halves = [(0, 4), (4, 7)]

    with TileContext(nc) as tc:
        with (
            tc.tile_pool(name="const", bufs=1) as cpool,
            tc.tile_pool(name="io", bufs=3) as iopool,
            tc.tile_pool(name="work", bufs=2) as wpool,
            tc.tile_pool(name="psum", bufs=2, space="PSUM") as ppool,
        ):
            ident32 = cpool.tile([128, 128], fp32, tag="id32")
            nc.sync.dma_start(out=ident32[:, :], in_=id32[:, :])
            ident16 = cpool.tile([128, 128], fp16, tag="id16")
            nc.sync.dma_start(out=ident16[:, :], in_=id16[:, :])
            g1_t = cpool.tile([128, 128], fp32, tag="g1")
            nc.sync.dma_start(out=g1_t[:, :], in_=g1[:, :])
            w2_t = cpool.tile([128, 128], fp16, tag="w2")
            nc.sync.dma_start(out=w2_t[:, :], in_=w2[:, :])
            qinv_t = cpool.tile([128, 1], fp32, tag="qinv")
            nc.sync.dma_start(out=qinv_t[:, :], in_=qinv[:, :])
            cmagic_t = cpool.tile([128, 1], fp32, tag="cmagic")
            nc.sync.dma_start(out=cmagic_t[:, :], in_=cmagic[:, :])

            for n in range(N_SHARD):
                x_flat = xs[n, :, :, :].rearrange("c h w -> c (h w)")
                y_flat = ys[n, :, :, :].rearrange("c h w -> c (h w)")
                for hb in range(HB):
                    xt = iopool.tile([128, ROWCHUNK], fp32, tag="xt")
                    nc.sync.dma_start(
                        out=xt[:, :],
                        in_=x_flat[:, hb * ROWCHUNK : (hb + 1) * ROWCHUNK],
                    )
                    yt = iopool.tile([128, ROWCHUNK], fp32, tag="yt")
                    # gather/scatter views: f = j*112 + wb*8 + k -> dims (wb, j, k)
                    xt_g = xt[:, :].rearrange("c (j wb k) -> c wb j k", j=8, wb=WB, k=8)
                    yt_g = yt[:, :].rearrange("c (j wb k) -> c wb j k", j=8, wb=WB, k=8)
                    # pre-gather to block-contiguous layout: f' = wb*64 + j*8 + k
                    # (matmul weights APs must be single-free-dim, so the
                    # gather cannot ride on the T1 transpose itself)
                    xg = wpool.tile([128, ROWCHUNK], fp32, tag="xg")
                    xg_g = xg[:, :].rearrange("c (wb j k) -> c wb j k", wb=WB, j=8, k=8)
                    nc.vector.tensor_copy(xg_g, xt_g)

                    for w0, w1 in halves:
                        nw = w1 - w0
                        ncols = nw * 128
                        # --- T1: blocks -> Kron layout [(s,j,k), c] ---
                        t1p = ppool.tile([128, 512], fp32, tag="t1")
                        for ii, wbp in enumerate(range(w0, w1)):
                            nc.tensor.transpose(
                                t1p[:, ii * 128 : (ii + 1) * 128],
                                xg[:, wbp * 128 : (wbp + 1) * 128],
                                ident32[:, :],
                            )
                        kt = wpool.tile([128, 512], fp32, tag="kt")
                        nc.scalar.copy(kt[:, :ncols], t1p[:, :ncols])
                        # --- MM1: forward 2D DCT (fp32) ---
                        mm1p = ppool.tile([128, 512], fp32, tag="mm1")
                        nc.tensor.matmul(
                            mm1p[:, :ncols], g1_t[:, :], kt[:, :ncols],
                            start=True, stop=True,
                        )
                        # --- quantize: round(X/q) via magic-number add ---
                        rt1 = wpool.tile([128, 512], fp32, tag="rt1")
                        nc.scalar.activation(
                            rt1[:, :ncols], mm1p[:, :ncols],
                            mybir.ActivationFunctionType.Identity,
                            bias=cmagic_t[:, :], scale=qinv_t[:, :],
                        )
                        rt2 = wpool.tile([128, 512], fp16, tag="rt2")
                        nc.vector.tensor_scalar_sub(rt2[:, :ncols], rt1[:, :ncols], CMAGIC)
                        # --- MM2: dequant + inverse 2D DCT (fp16) ---
                        mm2p = ppool.tile([128, 512], fp32, tag="mm2")
                        nc.tensor.matmul(
                            mm2p[:, :ncols], w2_t[:, :], rt2[:, :ncols],
                            start=True, stop=True,
                        )
                        y16 = wpool.tile([128, 512], fp16, tag="y16")
                        nc.scalar.copy(y16[:, :ncols], mm2p[:, :ncols])
                        # --- T2: back to [c, (s,j,k)] ---
                        t2p = ppool.tile([128, 512], fp16, tag="t2")
                        for ii, wbp in enumerate(range(w0, w1)):
                            nc.tensor.transpose(
                                t2p[:, ii * 128 : (ii + 1) * 128],
                                y16[:, ii * 128 : (ii + 1) * 128],
                                ident16[:, :],
                            )
                        # --- C3: scatter (s,j,k) -> natural (h,w) layout ---
                        for ii, wbp in enumerate(range(w0, w1)):
                            nc.vector.tensor_copy(
                                yt_g[:, 2 * wbp : 2 * wbp + 2, :, :],
                                t2p[:, ii * 128 : (ii + 1) * 128],
                            )
                    nc.sync.dma_start(
                        out=y_flat[:, hb * ROWCHUNK : (hb + 1) * ROWCHUNK],
                        in_=yt[:, :],
                    )
    return nc


_PROGRAM = None


def kernel(x: np.ndarray, q_table: np.ndarray) -> np.ndarray:
    global _PROGRAM, LAST_EXEC_NS, LAST_PROFILE
    from concourse.bass_utils import run_bass_kernel_spmd

    x = np.ascontiguousarray(np.asarray(x, dtype=np.float32))
    q_table = np.asarray(q_table, dtype=np.float32)
    assert x.shape == (N_FULL, C, H, W), x.shape

    G1, W2, qinv, I32, I16 = _build_consts(q_table)
    if _PROGRAM is None:
        nc = _build_program()
        nc.finalize()
        _PROGRAM = nc
    nc = _PROGRAM

    core_ids = list(range(N_CORES))
    in_maps = []
    for i in core_ids:
        shard = np.ascontiguousarray(x[i * N_SHARD : (i + 1) * N_SHARD])
        in_maps.append(
            {"x": shard, "g1": G1, "w2": W2, "qinv": qinv, "id32": I32, "id16": I16,
             "cmagic": np.full((128, 1), CMAGIC, np.float32)}
        )

    res = run_bass_kernel_spmd(nc, in_maps, core_ids)
    LAST_EXEC_NS = res.exec_time_ns
    LAST_PROFILE = res.profile_json
    out = np.concatenate([np.asarray(res.results[i]["y"]) for i in core_ids], axis=0)
    return out.astype(np.float32)
